# revision 6
# baseline (speedup 1.0000x reference)
"""Trainium2 Bass kernel for the YAT MixerBlock (nn_MixerBlock_12524124635797).

Data-parallel over batch (64 -> 8 per core); all large GEMMs in fp8e4
DoubleRow mode (K=256 per instruction, ~1.9x PE throughput vs fp16).

Numerator and denominator share one PSUM accumulation: the dot GEMM uses
stationary fp8(-2*w), so psum = -2*dot. ACT reads the raw psum to form the
squared numerator ((scale*psum + bias)^2, scale = -0.5*sqrt(F)); then a tiny
fp16 matmul accumulates wn+eps+xn into the same psum (start=False), turning
it into the denominator, read directly by the DVE reciprocal. This removes
the per-element affine op entirely.

Scale folding (powers of two; alpha rides in the ACT scale/bias tensors):
  h1' = 64*s_t^a*h1 (fp8), w2 plain fp8, shortcut identity*64, copy*(1/64)
  h2' = 128*s_c^a*h2 (fp8), w4*16 fp8, bias*2048, final unscale 2^-11
Output is fp16 (ample vs the 2e-2 gate), halving the output DMA.
"""

import numpy as np
import ml_dtypes

import concourse.bass as bass
import concourse.bacc as bacc
import concourse.mybir as mybir
from concourse import bass_utils
from concourse import tile

F8 = mybir.dt.float8e4
F16 = mybir.dt.float16
F32 = mybir.dt.float32
BF16 = mybir.dt.bfloat16
AF = mybir.ActivationFunctionType
DR = mybir.MatmulPerfMode.DoubleRow
ALU = mybir.AluOpType
NP8 = ml_dtypes.float8_e4m3

EPS = 0.1
B, P, C, T, M3 = 64, 196, 768, 384, 3072
NCORES = 8
BL = B // NCORES          # 8 batches per core
ROWS = BL * P             # 1568 rows per core
ROWSP = 1664              # ROWS padded to a multiple of 128
RB = 256                  # channel row-block


def build_program():
    nc = bacc.Bacc(
        "TRN2",
        target_bir_lowering=False,
        debug=False,
        enable_asserts=False,
        num_devices=NCORES,
    )

    d = {}
    d["x16"] = nc.dram_tensor("x16", [BL, 128, 2, C], F16, kind="ExternalInput").ap()
    d["x8"] = nc.dram_tensor("x8", [BL, 128, 2, C], F8, kind="ExternalInput").ap()
    d["twm2"] = nc.dram_tensor("twm2", [128, 2, T], F8, kind="ExternalInput").ap()
    d["w28"] = nc.dram_tensor("w28", [128, 3, P], F8, kind="ExternalInput").ap()
    d["i196"] = nc.dram_tensor("i196", [128, 2, P], F16, kind="ExternalInput").ap()
    d["b2r"] = nc.dram_tensor("b2r", [1, P], F16, kind="ExternalInput").ap()
    d["augwt"] = nc.dram_tensor("augwt", [1, 3, 128], F16, kind="ExternalInput").ap()
    d["cwm2"] = nc.dram_tensor("cwm2", [128, 6, M3], F8, kind="ExternalInput").ap()
    d["w48"] = nc.dram_tensor("w48", [128, 24, C], F8, kind="ExternalInput").ap()
    d["b4r"] = nc.dram_tensor("b4r", [1, C], F16, kind="ExternalInput").ap()
    d["augwc"] = nc.dram_tensor("augwc", [2, 24, 128], F16, kind="ExternalInput").ap()
    d["tbsc"] = nc.dram_tensor("tbsc", [128, 3], F32, kind="ExternalInput").ap()
    d["cbsc"] = nc.dram_tensor("cbsc", [128, 24], F32, kind="ExternalInput").ap()
    d["sqsc"] = nc.dram_tensor("sqsc", [128, 2], F32, kind="ExternalInput").ap()
    out_dram = nc.dram_tensor("out", [ROWS, C], F16, kind="ExternalOutput").ap()

    blocks = []
    r0 = 0
    while r0 < ROWS:
        rn = min(RB, ROWS - r0)
        blocks.append((r0, rn, (r0 + rn - 1) // P))
        r0 += rn

    with tile.TileContext(nc) as tc:
        with tc.tile_pool(name="consts", bufs=1) as cp:
            twm2 = cp.tile([128, 2, T], F8)
            w28 = cp.tile([128, 3, P], F8)
            i196 = cp.tile([128, 2, P], F16)
            b2r = cp.tile([128, P], F16)
            augwt = cp.tile([1, 3, 128], F16)
            cwm2 = cp.tile([128, 6, M3], F8)
            w48 = cp.tile([128, 24, C], F8)
            b4r = cp.tile([128, C], F16)
            augwc = cp.tile([2, 24, 128], F16)
            tbsc = cp.tile([128, 3], F32)
            cbsc = cp.tile([128, 24], F32)
            sqsc = cp.tile([128, 2], F32)
            ones = cp.tile([128, 128], F16)
            onesr = cp.tile([1, RB], F16)
            augmc = cp.tile([2, ROWSP], F16)     # row0=ones, row1=xn2
            x2T = cp.tile([128, 6, ROWSP], F16)
            x2T8 = cp.tile([128, 6, ROWSP], F8)
            x2sq = cp.tile([128, 6, ROWSP], F16)

            # startup DMAs: only batch 0/1 x plus token consts up front on
            # sync; later batches stream in during the token stage; channel
            # weights on the scalar/gpsimd queues.
            nc.sync.dma_start(twm2[:], d["twm2"])
            x16s = [cp.tile([128, 2, C], F16, name=f"x16_{b}") for b in range(BL)]
            x8s = [cp.tile([128, 2, C], F8, name=f"x8_{b}") for b in range(BL)]

            def load_x(b):
                nc.sync.dma_start(x8s[b][:], d["x8"][b])
                nc.sync.dma_start(x16s[b][:], d["x16"][b])

            load_x(0)
            nc.sync.dma_start(w28[:], d["w28"])
            nc.sync.dma_start(augwt[:], d["augwt"])
            nc.sync.dma_start(tbsc[:], d["tbsc"])
            nc.sync.dma_start(sqsc[:], d["sqsc"])
            load_x(1)
            nc.sync.dma_start(i196[:], d["i196"])
            nc.sync.dma_start(b2r[0:1, :], d["b2r"])
            nc.sync.dma_start(augwc[:], d["augwc"])
            nc.sync.dma_start(cbsc[:], d["cbsc"])
            nc.sync.dma_start(b4r[0:1, :], d["b4r"])
            nc.scalar.dma_start(cwm2[:], d["cwm2"])
            nc.gpsimd.dma_start(w48[:], d["w48"])
            nc.vector.memset(ones[:], 1.0)
            nc.vector.memset(onesr[:], 1.0)
            nc.vector.memset(augmc[0:1, :], 1.0)
            nc.vector.memset(x2T[:, :, ROWS:ROWSP], 0.0)

            with (
                tc.tile_pool(name="work", bufs=2) as wp,
                tc.tile_pool(name="psum", bufs=1, space="PSUM") as pp,
            ):
                h1s = {}

                def token_a(b):
                    """dot1 + yat chain -> h1 (fp8)."""
                    xb16, xb8 = x16s[b], x8s[b]
                    if b + 2 < BL:
                        load_x(b + 2)
                    xsq = wp.tile([128, 2, C], F16, tag="xsq")
                    nc.vector.tensor_mul(xsq[:], xb16[:], xb16[:])
                    # x-norm row (into a ps slot, then fp16 at partition 0)
                    psn = pp.tile([128, 3, RB], F32, tag="ps", bufs=2)
                    for q in range(3):
                        for kc in range(2):
                            nc.tensor.matmul(
                                psn[0:1, q, :], ones[:, 0:1],
                                xsq[:, kc, q * 256 : (q + 1) * 256],
                                start=(kc == 0), stop=(kc == 1),
                            )
                    xn16 = wp.tile([1, C], F16, tag="xn16")
                    nc.scalar.copy(xn16[0:1, :], psn[0:1, 0:3, :])

                    h1 = wp.tile([128, 3, C], F8, tag="h1")
                    h1s[b] = h1

                    def tok_tail(q, ps, sq):
                        c0 = q * 256
                        for tcn in range(3):
                            nc.tensor.matmul(
                                ps[:, tcn, :], augwt[0:1, tcn, :],
                                onesr[0:1, :],
                                start=False, stop=False,
                                skip_group_check=True,
                            )
                            nc.tensor.matmul(
                                ps[:, tcn, :], ones[0:1, :],
                                xn16[0:1, c0 : c0 + 256],
                                start=False, stop=True,
                                skip_group_check=True,
                            )
                        rec = wp.tile([128, 3, RB], F32, tag="rec")
                        nc.vector.reciprocal_approx_fast(rec[:], ps[:, 0:3, :])
                        nc.gpsimd.tensor_mul(h1[:, :, c0 : c0 + 256], sq[:], rec[:])

                    pend = None
                    for q in range(3):
                        c0 = q * 256
                        ps = pp.tile([128, 3, RB], F32, tag="ps", bufs=2)
                        for tcn in range(3):
                            nc.tensor.matmul(
                                ps[:, tcn, :],
                                twm2[:, :, tcn * 128 : (tcn + 1) * 128],
                                xb8[:, :, c0 : c0 + 256],
                                start=True, stop=True, perf_mode=DR,
                            )
                        sq = wp.tile([128, 3, RB], BF16, tag="sq")
                        for tcn in range(3):
                            nc.scalar.activation(
                                sq[:, tcn, :], ps[:, tcn, :], AF.Square,
                                bias=tbsc[:, tcn : tcn + 1],
                                scale=sqsc[:, 0:1],
                            )
                        if pend is not None:
                            tok_tail(*pend)
                        pend = (q, ps, sq)
                    tok_tail(*pend)

                def token_b(b):
                    """token linear + shortcut + bias -> x2T / x2T8 / x2sq."""
                    r0 = b * P
                    xb16, h1 = x16s[b], h1s.pop(b)
                    for g in range(3):
                        px = pp.tile([128, 3, RB], F32, tag="ps", bufs=2)
                        for j in range(2):
                            mc = g * 2 + j
                            ms = slice(mc * 128, (mc + 1) * 128)
                            nc.tensor.matmul(
                                px[:, j, 0:P], h1[:, 0:2, ms], w28[:, 0:2, :],
                                start=True, stop=False, perf_mode=DR,
                            )
                            nc.tensor.matmul(
                                px[:, j, 0:P], h1[:, 2, ms], w28[:, 2, :],
                                start=False, stop=False,
                            )
                            for kc, kn in ((0, 128), (1, 68)):
                                nc.tensor.matmul(
                                    px[:, j, 0:P], xb16[0:kn, kc, ms],
                                    i196[0:kn, kc, :],
                                    start=False, stop=False,
                                )
                            nc.tensor.matmul(
                                px[:, j, 0:P], ones[0:1, :], b2r[0:1, :],
                                start=False, stop=True,
                            )
                        for j in range(2):
                            mc = g * 2 + j
                            nc.scalar.activation(
                                x2T[:, mc, r0 : r0 + P], px[:, j, 0:P], AF.Copy,
                                scale=1.0 / 64.0,
                            )
                    nc.vector.tensor_mul(
                        x2sq[:, :, r0 : r0 + P],
                        x2T[:, :, r0 : r0 + P], x2T[:, :, r0 : r0 + P],
                    )
                    nc.vector.tensor_copy(
                        x2T8[:, :, r0 : r0 + P], x2T[:, :, r0 : r0 + P]
                    )

                def chan_norm(bi):
                    """row norms for block bi -> augmc row 1 (early)."""
                    r0, rn, _ = blocks[bi]
                    psx = pp.tile([128, 3, RB], F32, tag="ps", bufs=2)
                    for kc in range(6):
                        nc.tensor.matmul(
                            psx[0:1, 0, 0:rn], ones[:, 0:1],
                            x2sq[:, kc, r0 : r0 + rn],
                            start=(kc == 0), stop=(kc == 5),
                        )
                    xn16c = wp.tile([1, RB], F16, tag="xn16c")
                    nc.scalar.copy(xn16c[0:1, 0:rn], psx[0:1, 0, 0:rn])
                    nc.sync.dma_start(
                        augmc[1:2, r0 : r0 + rn], xn16c[0:1, 0:rn]
                    )

                def chan_block(bi):
                    r0, rn, _ = blocks[bi]
                    nsub = (rn + 127) // 128
                    po = pp.tile([128, 2, C], F32, tag="po", bufs=1)
                    stage = []  # software pipeline: h2w4 lags 2 groups

                    def h2w4(g, h2):
                        for s in range(nsub):
                            sn = min(128, rn - s * 128)
                            for no in range(0, C, 256):
                                nc.tensor.matmul(
                                    po[0:sn, s, no : no + 256],
                                    h2[:, 0:2, s * 128 : s * 128 + sn],
                                    w48[:, 2 * g : 2 * g + 2, no : no + 256],
                                    start=(g == 0), stop=False, perf_mode=DR,
                                )

                    def ch_tail(g, pd, sq2):
                        for j in range(2):
                            mc = g * 2 + j
                            nc.tensor.matmul(
                                pd[:, j, 0:rn], augwc[0:2, mc, :],
                                augmc[0:2, r0 : r0 + rn],
                                start=False, stop=True, skip_group_check=True,
                            )
                        rec2 = wp.tile([128, 2, RB], F32, tag="rec2", bufs=3)
                        nc.vector.reciprocal_approx_fast(
                            rec2[:, 0:2, 0:rn], pd[:, 0:2, 0:rn]
                        )
                        h2 = wp.tile([128, 2, RB], F8, tag="h2", bufs=3)
                        nc.gpsimd.tensor_mul(
                            h2[:, 0:2, 0:rn], sq2[:, 0:2, 0:rn], rec2[:, 0:2, 0:rn]
                        )
                        stage.append((g, h2))

                    pend = None
                    for g in range(12):
                        pd = pp.tile([128, 3, RB], F32, tag="ps", bufs=2)
                        for j in range(2):
                            mc = g * 2 + j
                            ms = slice(mc * 128, (mc + 1) * 128)
                            for kcp in range(3):
                                nc.tensor.matmul(
                                    pd[:, j, 0:rn],
                                    cwm2[:, 2 * kcp : 2 * kcp + 2, ms],
                                    x2T8[:, 2 * kcp : 2 * kcp + 2, r0 : r0 + rn],
                                    start=(kcp == 0), stop=(kcp == 2),
                                    perf_mode=DR,
                                )
                        sq2 = wp.tile([128, 2, RB], BF16, tag="sq2", bufs=3)
                        for j in range(2):
                            mc = g * 2 + j
                            nc.scalar.activation(
                                sq2[:, j, 0:rn], pd[:, j, 0:rn], AF.Square,
                                bias=cbsc[:, mc : mc + 1], scale=sqsc[:, 1:2],
                            )
                        if pend is not None:
                            ch_tail(*pend)
                        pend = (g, pd, sq2)
                        if len(stage) > 1:
                            h2w4(*stage.pop(0))
                    ch_tail(*pend)
                    while stage:
                        h2w4(*stage.pop(0))

                    qeng = [nc.sync, nc.scalar]
                    for s in range(nsub):
                        sn = min(128, rn - s * 128)
                        rs = r0 + s * 128
                        for no in range(0, C, 512):
                            nn = min(512, C - no)
                            nc.tensor.matmul(
                                po[0:sn, s, no : no + nn],
                                ones[0:1, 0:sn], b4r[0:1, no : no + nn],
                                start=False, stop=True,
                            )
                        x2row = wp.tile([128, 6, 128], F16, tag="x2row", bufs=3)
                        for kc in range(6):
                            qeng[kc % 2].dma_start_transpose(
                                x2row[:, kc, :], x2T[:, kc, rs : rs + 128]
                            )
                        osb = wp.tile([128, C], F16, tag="osb", bufs=3)
                        nc.vector.scalar_tensor_tensor(
                            osb[0:sn, :], po[0:sn, s, :], 2.0 ** -11,
                            x2row[0:sn, :, :].rearrange("p a b -> p (a b)"),
                            ALU.mult, ALU.add,
                        )
                        nc.sync.dma_start(out_dram[rs : rs + sn, :], osb[0:sn, :])

                # emission order: stagger token A/B, channel norms early,
                # channel blocks as soon as their batches are done.
                emitted_b = -1
                next_norm = 0
                next_blk = 0

                def after_b(b):
                    nonlocal next_norm, next_blk
                    while next_norm < len(blocks) and blocks[next_norm][2] <= b:
                        chan_norm(next_norm)
                        next_norm += 1

                def blocks_ready(b):
                    nonlocal next_blk
                    while next_blk < len(blocks) and blocks[next_blk][2] <= b:
                        chan_block(next_blk)
                        next_blk += 1

                token_a(0)
                for b in range(BL):
                    if b + 1 < BL:
                        token_a(b + 1)
                    token_b(b)
                    after_b(b)
                    if b >= 1:
                        blocks_ready(b - 1)
                blocks_ready(BL - 1)

    nc.compile()
    return nc


def _pack_kpn8(w, n_chunks, scale):
    """(K, N) fp32 -> (128, n_chunks, N) fp8 with zero K-padding."""
    k, n = w.shape
    out = np.zeros((n_chunks * 128, n), np.float32)
    out[:k] = w * scale
    return np.ascontiguousarray(
        out.reshape(n_chunks, 128, n).transpose(1, 0, 2)
    ).astype(NP8)


def _pack_col(v, n_chunks):
    out = np.zeros((n_chunks * 128,), np.float32)
    out[: v.shape[0]] = v
    return np.ascontiguousarray(out.reshape(n_chunks, 128).T)


_PROGRAM = None


def _get_program():
    global _PROGRAM
    if _PROGRAM is None:
        _PROGRAM = build_program()
    return _PROGRAM


def kernel(x, tw, tb, t_alpha, w2, b2, cw, cb, c_alpha, w4, b4, _trace=False):
    x = np.asarray(x, np.float32)
    tw = np.asarray(tw, np.float32)
    tb = np.asarray(tb, np.float32)
    w2 = np.asarray(w2, np.float32)
    b2 = np.asarray(b2, np.float32)
    cw = np.asarray(cw, np.float32)
    cb = np.asarray(cb, np.float32)
    w4 = np.asarray(w4, np.float32)
    b4 = np.asarray(b4, np.float32)

    s_t = np.float32(np.sqrt(np.float32(T / np.log(T + 1.0)))) ** np.asarray(
        t_alpha, np.float32
    )[0]
    s_c = np.float32(np.sqrt(np.float32(M3 / np.log(M3 + 1.0)))) ** np.asarray(
        c_alpha, np.float32
    )[0]
    g_t = np.float32(np.sqrt(64.0 * s_t))
    g_c = np.float32(np.sqrt(128.0 * s_c))

    wn_t = (tw ** 2).sum(1) + EPS
    wn_c = (cw ** 2).sum(1) + EPS
    augwt = np.zeros((1, 3, 128), np.float16)
    augwt[0].flat[:T] = wn_t.astype(np.float16)
    augwc = np.zeros((2, 24, 128), np.float16)
    augwc[0].flat[:M3] = wn_c.astype(np.float16)
    augwc[1] = 1.0
    sqsc = np.zeros((128, 2), np.float32)
    sqsc[:, 0] = -0.5 * g_t
    sqsc[:, 1] = -0.5 * g_c

    shared = {
        "twm2": _pack_kpn8(tw.T, 2, -2.0),
        "w28": _pack_kpn8(w2.T, 3, 1.0),
        "i196": np.ascontiguousarray(
            np.pad(64.0 * np.eye(P, dtype=np.float32), ((0, 60), (0, 0)))
            .reshape(2, 128, P).transpose(1, 0, 2)).astype(np.float16),
        "b2r": (64.0 * b2).astype(np.float16).reshape(1, P),
        "augwt": augwt,
        "cwm2": _pack_kpn8(cw.T, 6, -2.0),
        "w48": _pack_kpn8(w4.T, 24, 16.0),
        "b4r": (2048.0 * b4).astype(np.float16).reshape(1, C),
        "augwc": augwc,
        "tbsc": _pack_col(g_t * tb, 3),
        "cbsc": _pack_col(g_c * cb, 24),
        "sqsc": sqsc,
    }
    xr = x.reshape(NCORES, BL, P, C)
    x16 = np.zeros((NCORES, BL, 128, 2, C), np.float16)
    x16[:, :, :, 0] = xr[:, :, 0:128]
    x16[:, :, 0:68, 1] = xr[:, :, 128:P]
    x8 = np.zeros((NCORES, BL, 128, 2, C), NP8)
    x8[:, :, :, 0] = xr[:, :, 0:128].astype(NP8)
    x8[:, :, 0:68, 1] = xr[:, :, 128:P].astype(NP8)
    in_maps = [dict(shared, x16=x16[c], x8=x8[c]) for c in range(NCORES)]

    nc = _get_program()
    kwargs = {}
    if _trace:
        import os
        import shutil

        shutil.rmtree("/tmp/bass_ntff", ignore_errors=True)
        os.makedirs("/tmp/bass_ntff", exist_ok=True)
        kwargs["tmpdir"] = "/tmp/bass_ntff"
    res = bass_utils.run_bass_kernel_spmd(
        nc, in_maps, core_ids=list(range(NCORES)), trace=_trace, **kwargs
    )
    out = np.concatenate(
        [np.asarray(res.results[c]["out"]) for c in range(NCORES)], axis=0
    )
    out = out.reshape(B, P, C).astype(np.float32)
    if _trace:
        kernel.last_results = res
    return out


# revision 13
# speedup vs baseline: 1.0028x; 1.0028x over previous
"""Trainium2 Bass kernel for the YAT MixerBlock (nn_MixerBlock_12524124635797).

Data-parallel over batch (64 -> 8 per core); all large GEMMs in fp8e4
DoubleRow mode (K=256 per instruction, ~1.9x PE throughput vs fp16), with
N=512 output columns per instruction so LDWEIGHTS fully amortizes.

Numerator and denominator share one PSUM accumulation: the dot GEMM uses
stationary fp8(-2*w), so psum = -2*dot. ACT reads the raw psum to form the
squared numerator ((scale*psum + bias)^2, scale = -0.5*sqrt(F)); then a tiny
K=2 fp16 matmul accumulates wn+eps+xn into the same psum (start=False),
turning it into the denominator, read directly by the DVE reciprocal.

Scale folding (powers of two; alpha rides in the ACT scale/bias tensors):
  h1' = 64*s_t^a*h1 (fp8), w2 plain fp8, shortcut identity*64, copy*(1/64)
  h2' = 128*s_c^a*h2 (fp8), w4*16 fp8, bias*2048, final unscale 2^-11
Output is fp16 (ample vs the 2e-2 gate), halving the output DMA.
"""

import numpy as np
import ml_dtypes

import concourse.bass as bass
import concourse.bacc as bacc
import concourse.mybir as mybir
from concourse import bass_utils
from concourse import tile

F8 = mybir.dt.float8e4
F16 = mybir.dt.float16
F32 = mybir.dt.float32
BF16 = mybir.dt.bfloat16
AF = mybir.ActivationFunctionType
DR = mybir.MatmulPerfMode.DoubleRow
ALU = mybir.AluOpType
NP8 = ml_dtypes.float8_e4m3

EPS = 0.1
B, P, C, T, M3 = 64, 196, 768, 384, 3072
NCORES = 8
BL = B // NCORES          # 8 batches per core
ROWS = BL * P             # 1568 rows per core
ROWSP = 1664              # ROWS padded to a multiple of 128
RB = 512                  # channel row-block
CH = C // 2               # half of C for the two h2w4 passes


def build_program():
    nc = bacc.Bacc(
        "TRN2",
        target_bir_lowering=False,
        debug=False,
        enable_asserts=False,
        num_devices=NCORES,
    )

    d = {}
    d["x16"] = nc.dram_tensor("x16", [BL, 128, 2, C], F16, kind="ExternalInput").ap()
    d["x8"] = nc.dram_tensor("x8", [BL, 128, 2, C], F8, kind="ExternalInput").ap()
    d["twm2"] = nc.dram_tensor("twm2", [128, 2, T], F8, kind="ExternalInput").ap()
    d["w28"] = nc.dram_tensor("w28", [128, 3, P], F8, kind="ExternalInput").ap()
    d["i196"] = nc.dram_tensor("i196", [128, 2, P], F16, kind="ExternalInput").ap()
    d["b2r"] = nc.dram_tensor("b2r", [1, P], F16, kind="ExternalInput").ap()
    d["augwt"] = nc.dram_tensor("augwt", [2, 3, 128], F16, kind="ExternalInput").ap()
    d["cwm2"] = nc.dram_tensor("cwm2", [128, 6, M3], F8, kind="ExternalInput").ap()
    d["w48"] = nc.dram_tensor("w48", [128, 24, C], F8, kind="ExternalInput").ap()
    d["b4r"] = nc.dram_tensor("b4r", [1, C], F16, kind="ExternalInput").ap()
    d["augwc"] = nc.dram_tensor("augwc", [2, 24, 128], F16, kind="ExternalInput").ap()
    d["tbsc"] = nc.dram_tensor("tbsc", [128, 3], F32, kind="ExternalInput").ap()
    d["cbsc"] = nc.dram_tensor("cbsc", [128, 24], F32, kind="ExternalInput").ap()
    d["sqsc"] = nc.dram_tensor("sqsc", [128, 2], F32, kind="ExternalInput").ap()
    out_dram = nc.dram_tensor("out", [ROWS, C], F16, kind="ExternalOutput").ap()

    blocks = []
    r0 = 0
    while r0 < ROWS:
        rn = min(RB, ROWS - r0)
        blocks.append((r0, rn, (r0 + rn - 1) // P))
        r0 += rn

    with tile.TileContext(nc) as tc:
        with tc.tile_pool(name="consts", bufs=1) as cp:
            twm2 = cp.tile([128, 2, T], F8)
            w28 = cp.tile([128, 3, P], F8)
            i196 = cp.tile([128, 2, P], F16)
            b2r = cp.tile([128, P], F16)
            augwt = cp.tile([2, 3, 128], F16)
            cwm2 = cp.tile([128, 6, M3], F8)
            w48 = cp.tile([128, 24, C], F8)
            b4r = cp.tile([128, C], F16)
            augwc = cp.tile([2, 24, 128], F16)
            tbsc = cp.tile([128, 3], F32)
            cbsc = cp.tile([128, 24], F32)
            sqsc = cp.tile([128, 2], F32)
            ones = cp.tile([128, 128], F16)
            augmt = cp.tile([2, C], F16)         # row0=ones, row1=xn (batch)
            augmc = cp.tile([2, ROWSP], F16)     # row0=ones, row1=xn2 (block)
            x2T = cp.tile([128, 6, ROWSP], F16)
            x2T8 = cp.tile([128, 6, ROWSP], F8)
            x2sq = cp.tile([128, 6, ROWSP], F16)

            # token-critical loads first; big channel weights are staged in
            # chunks from inside the token loop so they don't steal HBM
            # bandwidth from the x tiles at startup.
            nc.sync.dma_start(twm2[:], d["twm2"])
            x16s = [cp.tile([128, 2, C], F16, name=f"x16_{b}") for b in range(BL)]
            x8s = [cp.tile([128, 2, C], F8, name=f"x8_{b}") for b in range(BL)]

            def load_x(b):
                nc.sync.dma_start(x8s[b][:], d["x8"][b])
                nc.sync.dma_start(x16s[b][:], d["x16"][b])

            load_x(0)
            nc.sync.dma_start(w28[:], d["w28"])
            nc.sync.dma_start(augwt[:], d["augwt"])
            nc.sync.dma_start(tbsc[:], d["tbsc"])
            nc.sync.dma_start(sqsc[:], d["sqsc"])
            load_x(1)
            nc.sync.dma_start(i196[:], d["i196"])
            nc.sync.dma_start(b2r[0:1, :], d["b2r"])
            nc.sync.dma_start(augwc[:], d["augwc"])
            nc.sync.dma_start(cbsc[:], d["cbsc"])
            nc.sync.dma_start(b4r[0:1, :], d["b4r"])

            def stage_weights(step):
                # 2 chunks of cwm2 / w48 per token batch, done by b=3
                if step < 3:
                    nc.scalar.dma_start(
                        cwm2[:, 2 * step : 2 * step + 2, :],
                        d["cwm2"][:, 2 * step : 2 * step + 2, :],
                    )
                    nc.gpsimd.dma_start(
                        w48[:, 8 * step : 8 * step + 8, :],
                        d["w48"][:, 8 * step : 8 * step + 8, :],
                    )

            nc.vector.memset(ones[:], 1.0)
            nc.vector.memset(augmt[0:1, :], 1.0)
            nc.vector.memset(augmc[0:1, :], 1.0)
            nc.vector.memset(x2T[:, :, ROWS:ROWSP], 0.0)

            with (
                tc.tile_pool(name="work", bufs=2) as wp,
                tc.tile_pool(name="psum", bufs=1, space="PSUM") as pp,
            ):
                h1s = {}

                def ps_tile():
                    # shared psum tag: [128, 4, 256] fp32 = 2 banks; token
                    # views it as [128, 3, 256], the channel as two 512-row
                    # halves via the flattened [128, 1024] AP.
                    return pp.tile([128, 4, 256], F32, tag="ps", bufs=2, name="ps")

                def token_a(b):
                    """dot1 + yat chain -> h1 (fp8)."""
                    xb16, xb8 = x16s[b], x8s[b]
                    if b + 2 < BL:
                        load_x(b + 2)
                    stage_weights(b)
                    xsq = wp.tile([128, 2, C], F16, tag="xsq")
                    nc.vector.tensor_mul(xsq[:], xb16[:], xb16[:])
                    # x-norm row at partition 0, then fp16, then DMA to
                    # partition 1 of augmt
                    psn = ps_tile()
                    pv = psn.rearrange("p a b -> p (a b)")
                    for q in range(3):
                        for kc in range(2):
                            nc.tensor.matmul(
                                pv[0:1, q * 256 : (q + 1) * 256], ones[:, 0:1],
                                xsq[:, kc, q * 256 : (q + 1) * 256],
                                start=(kc == 0), stop=(kc == 1),
                            )
                    xn16 = wp.tile([1, C], F16, tag="xn16")
                    nc.scalar.copy(xn16[0:1, :], pv[0:1, 0:C])
                    nc.gpsimd.dma_start(augmt[1:2, :], xn16[0:1, :])

                    h1 = wp.tile([128, 3, C], F8, tag="h1")
                    h1s[b] = h1

                    def tok_tail(q, ps, sq):
                        c0 = q * 256
                        for tcn in range(3):
                            nc.tensor.matmul(
                                ps[:, tcn, :], augwt[0:2, tcn, :],
                                augmt[0:2, c0 : c0 + 256],
                                start=False, stop=True,
                                skip_group_check=True,
                            )
                        rec = wp.tile([128, 3, 256], F32, tag="rec")
                        nc.vector.reciprocal_approx_fast(rec[:], ps[:, 0:3, :])
                        nc.gpsimd.tensor_mul(h1[:, :, c0 : c0 + 256], sq[:], rec[:])

                    pend = None
                    for q in range(3):
                        c0 = q * 256
                        ps = ps_tile()[:, 0:3, :]
                        for tcn in range(3):
                            nc.tensor.matmul(
                                ps[:, tcn, :],
                                twm2[:, :, tcn * 128 : (tcn + 1) * 128],
                                xb8[:, :, c0 : c0 + 256],
                                start=True, stop=True, perf_mode=DR,
                            )
                        sq = wp.tile([128, 3, 256], BF16, tag="sq")
                        for tcn in range(3):
                            nc.scalar.activation(
                                sq[:, tcn, :], ps[:, tcn, :], AF.Square,
                                bias=tbsc[:, tcn : tcn + 1],
                                scale=sqsc[:, 0:1],
                            )
                        if pend is not None:
                            tok_tail(*pend)
                        pend = (q, ps, sq)
                    tok_tail(*pend)

                def token_b(b):
                    """token linear + shortcut + bias -> x2T / x2T8 / x2sq."""
                    r0 = b * P
                    xb16, h1 = x16s[b], h1s.pop(b)
                    for g in range(3):
                        px = ps_tile()
                        for j in range(2):
                            mc = g * 2 + j
                            ms = slice(mc * 128, (mc + 1) * 128)
                            nc.tensor.matmul(
                                px[:, j, 0:P], h1[:, 0:2, ms], w28[:, 0:2, :],
                                start=True, stop=False, perf_mode=DR,
                            )
                            nc.tensor.matmul(
                                px[:, j, 0:P], h1[:, 2, ms], w28[:, 2, :],
                                start=False, stop=False,
                            )
                            for kc, kn in ((0, 128), (1, 68)):
                                nc.tensor.matmul(
                                    px[:, j, 0:P], xb16[0:kn, kc, ms],
                                    i196[0:kn, kc, :],
                                    start=False, stop=False,
                                )
                            nc.tensor.matmul(
                                px[:, j, 0:P], ones[0:1, :], b2r[0:1, :],
                                start=False, stop=True,
                            )
                        for j in range(2):
                            mc = g * 2 + j
                            nc.scalar.activation(
                                x2T[:, mc, r0 : r0 + P], px[:, j, 0:P], AF.Copy,
                                scale=1.0 / 64.0,
                            )
                    nc.vector.tensor_mul(
                        x2sq[:, :, r0 : r0 + P],
                        x2T[:, :, r0 : r0 + P], x2T[:, :, r0 : r0 + P],
                    )
                    nc.vector.tensor_copy(
                        x2T8[:, :, r0 : r0 + P], x2T[:, :, r0 : r0 + P]
                    )

                def chan_norm(bi):
                    """row norms for block bi -> augmc row 1 (early)."""
                    r0, rn, _ = blocks[bi]
                    psx = ps_tile().rearrange("p a b -> p (a b)")
                    for kc in range(6):
                        nc.tensor.matmul(
                            psx[0:1, 0:rn], ones[:, 0:1],
                            x2sq[:, kc, r0 : r0 + rn],
                            start=(kc == 0), stop=(kc == 5),
                        )
                    xn16c = wp.tile([1, RB], F16, tag="xn16c")
                    nc.scalar.copy(xn16c[0:1, 0:rn], psx[0:1, 0:rn])
                    nc.gpsimd.dma_start(
                        augmc[1:2, r0 : r0 + rn], xn16c[0:1, 0:rn]
                    )

                def chan_block(bi):
                    r0, rn, _ = blocks[bi]
                    nsub = (rn + 127) // 128
                    po = pp.tile([128, 4, CH], F32, tag="po", bufs=1)
                    h2all = wp.tile([128, 24, RB], F8, tag="h2all", bufs=1)

                    # shortcut transposes up front (x2T rows are complete)
                    qeng = [nc.sync, nc.scalar]
                    x2rows = []
                    for s in range(nsub):
                        x2row = wp.tile([128, 6, 128], F16, tag="x2row", bufs=4)
                        for kc in range(6):
                            qeng[kc % 2].dma_start_transpose(
                                x2row[:, kc, :],
                                x2T[:, kc, r0 + s * 128 : r0 + s * 128 + 128],
                            )
                        x2rows.append(x2row)

                    def ch_tail(g, pd, pdf, sq2):
                        for j in range(2):
                            mc = g * 2 + j
                            nc.tensor.matmul(
                                pdf[:, j * 512 : j * 512 + rn],
                                augwc[0:2, mc, :],
                                augmc[0:2, r0 : r0 + rn],
                                start=False, stop=True, skip_group_check=True,
                            )
                        rec2 = wp.tile([128, 2, RB], F32, tag="rec2", bufs=2)
                        if rn == RB:
                            nc.vector.reciprocal_approx_fast(
                                rec2.rearrange("p a b -> p (a b)"),
                                pdf[:, 0:1024],
                            )
                        else:
                            for j in range(2):
                                nc.vector.reciprocal_approx_fast(
                                    rec2[:, j, 0:rn],
                                    pdf[:, j * 512 : j * 512 + rn],
                                )
                        nc.gpsimd.tensor_mul(
                            h2all[:, 2 * g : 2 * g + 2, 0:rn],
                            sq2[:, 0:2, 0:rn], rec2[:, 0:2, 0:rn],
                        )

                    pend = None
                    for g in range(12):
                        pd = ps_tile()
                        pdf = pd.rearrange("p a b -> p (a b)")
                        for j in range(2):
                            mc = g * 2 + j
                            ms = slice(mc * 128, (mc + 1) * 128)
                            for kcp in range(3):
                                nc.tensor.matmul(
                                    pdf[:, j * 512 : j * 512 + rn],
                                    cwm2[:, 2 * kcp : 2 * kcp + 2, ms],
                                    x2T8[:, 2 * kcp : 2 * kcp + 2, r0 : r0 + rn],
                                    start=(kcp == 0), stop=(kcp == 2),
                                    perf_mode=DR,
                                )
                        sq2 = wp.tile([128, 2, RB], BF16, tag="sq2", bufs=3)
                        for j in range(2):
                            mc = g * 2 + j
                            nc.scalar.activation(
                                sq2[:, j, 0:rn],
                                pdf[:, j * 512 : j * 512 + rn], AF.Square,
                                bias=cbsc[:, mc : mc + 1], scale=sqsc[:, 1:2],
                            )
                        if pend is not None:
                            ch_tail(*pend)
                        pend = (g, pd, pdf, sq2)
                    ch_tail(*pend)

                    # two half-C h2w4 passes over the persistent h2
                    osbs = [wp.tile([128, C], F16, tag="osb", bufs=4, name="osb")
                            for _ in range(nsub)]
                    for half in range(2):
                        c0 = half * CH
                        for g in range(12):
                            for s in range(nsub):
                                sn = min(128, rn - s * 128)
                                nc.tensor.matmul(
                                    po[0:sn, s, :],
                                    h2all[:, 2 * g : 2 * g + 2,
                                          s * 128 : s * 128 + sn],
                                    w48[:, 2 * g : 2 * g + 2, c0 : c0 + CH],
                                    start=(g == 0), stop=False, perf_mode=DR,
                                )
                        for s in range(nsub):
                            sn = min(128, rn - s * 128)
                            rs = r0 + s * 128
                            nc.tensor.matmul(
                                po[0:sn, s, :], ones[0:1, 0:sn],
                                b4r[0:1, c0 : c0 + CH],
                                start=False, stop=True,
                            )
                            nc.vector.scalar_tensor_tensor(
                                osbs[s][0:sn, c0 : c0 + CH],
                                po[0:sn, s, :], 2.0 ** -11,
                                x2rows[s][0:sn, 3 * half : 3 * half + 3, :]
                                .rearrange("p a b -> p (a b)"),
                                ALU.mult, ALU.add,
                            )
                            if half == 1:
                                nc.sync.dma_start(
                                    out_dram[rs : rs + sn, :], osbs[s][0:sn, :]
                                )

                # emission order: stagger token A/B, channel norms early,
                # channel blocks as soon as their batches are done.
                next_norm = 0
                next_blk = 0

                def after_b(b):
                    nonlocal next_norm
                    while next_norm < len(blocks) and blocks[next_norm][2] <= b:
                        chan_norm(next_norm)
                        next_norm += 1

                def blocks_ready(b):
                    nonlocal next_blk
                    while next_blk < len(blocks) and blocks[next_blk][2] <= b:
                        chan_block(next_blk)
                        next_blk += 1

                token_a(0)
                for b in range(BL):
                    if b + 1 < BL:
                        token_a(b + 1)
                    token_b(b)
                    after_b(b)
                    if b >= 1:
                        blocks_ready(b - 1)
                blocks_ready(BL - 1)

    nc.compile()
    return nc


def _pack_kpn8(w, n_chunks, scale):
    """(K, N) fp32 -> (128, n_chunks, N) fp8 with zero K-padding."""
    k, n = w.shape
    out = np.zeros((n_chunks * 128, n), np.float32)
    out[:k] = w * scale
    return np.ascontiguousarray(
        out.reshape(n_chunks, 128, n).transpose(1, 0, 2)
    ).astype(NP8)


def _pack_col(v, n_chunks):
    out = np.zeros((n_chunks * 128,), np.float32)
    out[: v.shape[0]] = v
    return np.ascontiguousarray(out.reshape(n_chunks, 128).T)


_PROGRAM = None


def _get_program():
    global _PROGRAM
    if _PROGRAM is None:
        _PROGRAM = build_program()
    return _PROGRAM


def kernel(x, tw, tb, t_alpha, w2, b2, cw, cb, c_alpha, w4, b4, _trace=False):
    x = np.asarray(x, np.float32)
    tw = np.asarray(tw, np.float32)
    tb = np.asarray(tb, np.float32)
    w2 = np.asarray(w2, np.float32)
    b2 = np.asarray(b2, np.float32)
    cw = np.asarray(cw, np.float32)
    cb = np.asarray(cb, np.float32)
    w4 = np.asarray(w4, np.float32)
    b4 = np.asarray(b4, np.float32)

    s_t = np.float32(np.sqrt(np.float32(T / np.log(T + 1.0)))) ** np.asarray(
        t_alpha, np.float32
    )[0]
    s_c = np.float32(np.sqrt(np.float32(M3 / np.log(M3 + 1.0)))) ** np.asarray(
        c_alpha, np.float32
    )[0]
    g_t = np.float32(np.sqrt(64.0 * s_t))
    g_c = np.float32(np.sqrt(128.0 * s_c))

    wn_t = (tw ** 2).sum(1) + EPS
    wn_c = (cw ** 2).sum(1) + EPS
    augwt = np.zeros((2, 3, 128), np.float16)
    augwt[0].flat[:T] = wn_t.astype(np.float16)
    augwt[1] = 1.0
    augwc = np.zeros((2, 24, 128), np.float16)
    augwc[0].flat[:M3] = wn_c.astype(np.float16)
    augwc[1] = 1.0
    sqsc = np.zeros((128, 2), np.float32)
    sqsc[:, 0] = -0.5 * g_t
    sqsc[:, 1] = -0.5 * g_c

    shared = {
        "twm2": _pack_kpn8(tw.T, 2, -2.0),
        "w28": _pack_kpn8(w2.T, 3, 1.0),
        "i196": np.ascontiguousarray(
            np.pad(64.0 * np.eye(P, dtype=np.float32), ((0, 60), (0, 0)))
            .reshape(2, 128, P).transpose(1, 0, 2)).astype(np.float16),
        "b2r": (64.0 * b2).astype(np.float16).reshape(1, P),
        "augwt": augwt,
        "cwm2": _pack_kpn8(cw.T, 6, -2.0),
        "w48": _pack_kpn8(w4.T, 24, 16.0),
        "b4r": (2048.0 * b4).astype(np.float16).reshape(1, C),
        "augwc": augwc,
        "tbsc": _pack_col(g_t * tb, 3),
        "cbsc": _pack_col(g_c * cb, 24),
        "sqsc": sqsc,
    }
    xr = x.reshape(NCORES, BL, P, C)
    x16 = np.zeros((NCORES, BL, 128, 2, C), np.float16)
    x16[:, :, :, 0] = xr[:, :, 0:128]
    x16[:, :, 0:68, 1] = xr[:, :, 128:P]
    x8 = np.zeros((NCORES, BL, 128, 2, C), NP8)
    x8[:, :, :, 0] = xr[:, :, 0:128].astype(NP8)
    x8[:, :, 0:68, 1] = xr[:, :, 128:P].astype(NP8)
    in_maps = [dict(shared, x16=x16[c], x8=x8[c]) for c in range(NCORES)]

    nc = _get_program()
    kwargs = {}
    if _trace:
        import os
        import shutil

        shutil.rmtree("/tmp/bass_ntff", ignore_errors=True)
        os.makedirs("/tmp/bass_ntff", exist_ok=True)
        kwargs["tmpdir"] = "/tmp/bass_ntff"
    res = bass_utils.run_bass_kernel_spmd(
        nc, in_maps, core_ids=list(range(NCORES)), trace=_trace, **kwargs
    )
    out = np.concatenate(
        [np.asarray(res.results[c]["out"]) for c in range(NCORES)], axis=0
    )
    out = out.reshape(B, P, C).astype(np.float32)
    if _trace:
        kernel.last_results = res
    return out


# revision 16
# speedup vs baseline: 1.0401x; 1.0372x over previous
"""Trainium2 Bass kernel for the YAT MixerBlock (nn_MixerBlock_12524124635797).

Data-parallel over batch (64 -> 8 per core); all large GEMMs in fp8e4
DoubleRow mode (K=256 per instruction, ~1.9x PE throughput vs fp16), with
N=512 output columns per instruction so LDWEIGHTS fully amortizes.

Numerator and denominator share one PSUM accumulation: the dot GEMM uses
stationary fp8(-2*w), so psum = -2*dot. ACT reads the raw psum to form the
squared numerator ((scale*psum + bias)^2, scale = -0.5*sqrt(F)); then a tiny
K=2 fp16 matmul accumulates wn+eps+xn into the same psum (start=False),
turning it into the denominator, read directly by the DVE reciprocal.

Scale folding (powers of two; alpha rides in the ACT scale/bias tensors):
  h1' = 64*s_t^a*h1 (fp8), w2 plain fp8, shortcut identity*64, copy*(1/64)
  h2' = 128*s_c^a*h2 (fp8), w4*16 fp8, bias*2048, final unscale 2^-11
Output is fp16 (ample vs the 2e-2 gate), halving the output DMA.
"""

import numpy as np
import ml_dtypes

import concourse.bass as bass
import concourse.bacc as bacc
import concourse.mybir as mybir
from concourse import bass_utils
from concourse import tile

F8 = mybir.dt.float8e4
F16 = mybir.dt.float16
F32 = mybir.dt.float32
BF16 = mybir.dt.bfloat16
AF = mybir.ActivationFunctionType
DR = mybir.MatmulPerfMode.DoubleRow
ALU = mybir.AluOpType
NP8 = ml_dtypes.float8_e4m3

EPS = 0.1
B, P, C, T, M3 = 64, 196, 768, 384, 3072
NCORES = 8
BL = B // NCORES          # 8 batches per core
ROWS = BL * P             # 1568 rows per core
ROWSP = 1664              # ROWS padded to a multiple of 128
RB = 512                  # channel row-block
CH = C // 2               # half of C for the two h2w4 passes


def build_program():
    nc = bacc.Bacc(
        "TRN2",
        target_bir_lowering=False,
        debug=False,
        enable_asserts=False,
        num_devices=NCORES,
    )

    d = {}
    d["x16"] = nc.dram_tensor("x16", [BL, 128, 2, C], F16, kind="ExternalInput").ap()
    d["x8"] = nc.dram_tensor("x8", [BL, 128, 2, C], F8, kind="ExternalInput").ap()
    d["twm2"] = nc.dram_tensor("twm2", [128, 2, T], F8, kind="ExternalInput").ap()
    d["w28"] = nc.dram_tensor("w28", [128, 3, P], F8, kind="ExternalInput").ap()
    d["i196"] = nc.dram_tensor("i196", [128, 2, P], F16, kind="ExternalInput").ap()
    d["b2r"] = nc.dram_tensor("b2r", [1, P], F16, kind="ExternalInput").ap()
    d["augwt"] = nc.dram_tensor("augwt", [2, 3, 128], F16, kind="ExternalInput").ap()
    d["cwm2"] = nc.dram_tensor("cwm2", [128, 6, M3], F8, kind="ExternalInput").ap()
    d["w48"] = nc.dram_tensor("w48", [128, 24, C], F8, kind="ExternalInput").ap()
    d["b4r"] = nc.dram_tensor("b4r", [1, C], F16, kind="ExternalInput").ap()
    d["augwc"] = nc.dram_tensor("augwc", [2, 24, 128], F16, kind="ExternalInput").ap()
    d["tbsc"] = nc.dram_tensor("tbsc", [128, 3], F32, kind="ExternalInput").ap()
    d["cbsc"] = nc.dram_tensor("cbsc", [128, 24], F32, kind="ExternalInput").ap()
    d["sqsc"] = nc.dram_tensor("sqsc", [128, 2], F32, kind="ExternalInput").ap()
    out_dram = nc.dram_tensor("out", [ROWS, C], F16, kind="ExternalOutput").ap()

    blocks = []
    r0 = 0
    while r0 < ROWS:
        rn = min(RB, ROWS - r0)
        blocks.append((r0, rn, (r0 + rn - 1) // P))
        r0 += rn

    with tile.TileContext(nc) as tc:
        with tc.tile_pool(name="consts", bufs=1) as cp:
            twm2 = cp.tile([128, 2, T], F8)
            w28 = cp.tile([128, 3, P], F8)
            i196 = cp.tile([128, 2, P], F16)
            b2r = cp.tile([128, P], F16)
            augwt = cp.tile([2, 3, 128], F16)
            cwm2 = cp.tile([128, 6, M3], F8)
            w48 = cp.tile([128, 24, C], F8)
            b4r = cp.tile([128, C], F16)
            augwc = cp.tile([2, 24, 128], F16)
            tbsc = cp.tile([128, 3], F32)
            cbsc = cp.tile([128, 24], F32)
            sqsc = cp.tile([128, 2], F32)
            ones = cp.tile([128, 128], F16)
            augmt = cp.tile([2, C], F16)         # row0=ones, row1=xn (batch)
            augmc = cp.tile([2, ROWSP], F16)     # row0=ones, row1=xn2 (block)
            x2T = cp.tile([128, 6, ROWSP], F16)
            x2T8 = cp.tile([128, 6, ROWSP], F8)
            x2sq = cp.tile([128, 6, ROWSP], F16)

            # token-critical loads first; big channel weights are staged in
            # chunks from inside the token loop so they don't steal HBM
            # bandwidth from the x tiles at startup.
            nc.sync.dma_start(twm2[:], d["twm2"])
            x16s = [cp.tile([128, 2, C], F16, name=f"x16_{b}") for b in range(BL)]
            x8s = [cp.tile([128, 2, C], F8, name=f"x8_{b}") for b in range(BL)]

            def load_x(b):
                nc.sync.dma_start(x16s[b][:], d["x16"][b])
                nc.scalar.dma_start(x8s[b][:], d["x8"][b])

            load_x(0)
            nc.sync.dma_start(w28[:], d["w28"])
            nc.sync.dma_start(augwt[:], d["augwt"])
            nc.sync.dma_start(tbsc[:], d["tbsc"])
            nc.sync.dma_start(sqsc[:], d["sqsc"])
            load_x(1)
            nc.sync.dma_start(i196[:], d["i196"])
            nc.sync.dma_start(b2r[0:1, :], d["b2r"])
            nc.sync.dma_start(augwc[:], d["augwc"])
            nc.sync.dma_start(cbsc[:], d["cbsc"])
            nc.sync.dma_start(b4r[0:1, :], d["b4r"])

            def stage_weights(step):
                # cwm2 behind the x8 loads on scalar (steps 0-2); w48 on the
                # gpsimd queue a step later (first needed by block0's h2w4)
                if step < 3:
                    nc.scalar.dma_start(
                        cwm2[:, 2 * step : 2 * step + 2, :],
                        d["cwm2"][:, 2 * step : 2 * step + 2, :],
                    )
                if 1 <= step < 4:
                    nc.gpsimd.dma_start(
                        w48[:, 8 * (step - 1) : 8 * step, :],
                        d["w48"][:, 8 * (step - 1) : 8 * step, :],
                    )

            nc.vector.memset(ones[:], 1.0)
            nc.vector.memset(augmt[0:1, :], 1.0)
            nc.vector.memset(augmc[0:1, :], 1.0)
            nc.vector.memset(x2T[:, :, ROWS:ROWSP], 0.0)

            with (
                tc.tile_pool(name="work", bufs=2) as wp,
                tc.tile_pool(name="psum", bufs=1, space="PSUM") as pp,
            ):
                h1s = {}

                def ps_tile():
                    # shared psum tag: [128, 4, 256] fp32 = 2 banks; token
                    # views it as [128, 3, 256], the channel as two 512-row
                    # halves via the flattened [128, 1024] AP.
                    return pp.tile([128, 4, 256], F32, tag="ps", bufs=2, name="ps")

                def token_a(b):
                    """dot1 + yat chain -> h1 (fp8)."""
                    xb16, xb8 = x16s[b], x8s[b]
                    if b + 2 < BL:
                        load_x(b + 2)
                    stage_weights(b)
                    h1 = wp.tile([128, 3, C], F8, tag="h1")
                    h1s[b] = h1

                    def tok_tail(q, ps, sq):
                        c0 = q * 256
                        for tcn in range(3):
                            nc.tensor.matmul(
                                ps[:, tcn, :], augwt[0:2, tcn, :],
                                augmt[0:2, c0 : c0 + 256],
                                start=False, stop=True,
                                skip_group_check=True,
                            )
                        rec = wp.tile([128, 3, 256], F32, tag="rec")
                        nc.vector.reciprocal_approx_fast(rec[:], ps[:, 0:3, :])
                        nc.gpsimd.tensor_mul(h1[:, :, c0 : c0 + 256], sq[:], rec[:])

                    pend = None
                    for q in range(3):
                        c0 = q * 256
                        ps = ps_tile()[:, 0:3, :]
                        for tcn in range(3):
                            nc.tensor.matmul(
                                ps[:, tcn, :],
                                twm2[:, :, tcn * 128 : (tcn + 1) * 128],
                                xb8[:, :, c0 : c0 + 256],
                                start=True, stop=True, perf_mode=DR,
                            )
                        sq = wp.tile([128, 3, 256], BF16, tag="sq")
                        for tcn in range(3):
                            nc.scalar.activation(
                                sq[:, tcn, :], ps[:, tcn, :], AF.Square,
                                bias=tbsc[:, tcn : tcn + 1],
                                scale=sqsc[:, 0:1],
                            )
                        if q == 0:
                            # x-norm row (after q0 dots so the PE starts on
                            # x8 alone), then fp16 copy + DMA to augmt row 1
                            xsq = wp.tile([128, 2, C], F16, tag="xsq")
                            nc.vector.tensor_mul(xsq[:], xb16[:], xb16[:])
                            psn = ps_tile()
                            pv = psn.rearrange("p a b -> p (a b)")
                            for qq in range(3):
                                for kc in range(2):
                                    nc.tensor.matmul(
                                        pv[0:1, qq * 256 : (qq + 1) * 256],
                                        ones[:, 0:1],
                                        xsq[:, kc, qq * 256 : (qq + 1) * 256],
                                        start=(kc == 0), stop=(kc == 1),
                                    )
                            xn16 = wp.tile([1, C], F16, tag="xn16")
                            nc.scalar.copy(xn16[0:1, :], pv[0:1, 0:C])
                            nc.gpsimd.dma_start(augmt[1:2, :], xn16[0:1, :])
                        if pend is not None:
                            tok_tail(*pend)
                        pend = (q, ps, sq)
                    tok_tail(*pend)

                def token_b(b):
                    """token linear + shortcut + bias -> x2T / x2T8 / x2sq."""
                    r0 = b * P
                    xb16, h1 = x16s[b], h1s.pop(b)
                    for g in range(3):
                        px = ps_tile()
                        for j in range(2):
                            mc = g * 2 + j
                            ms = slice(mc * 128, (mc + 1) * 128)
                            nc.tensor.matmul(
                                px[:, j, 0:P], h1[:, 0:2, ms], w28[:, 0:2, :],
                                start=True, stop=False, perf_mode=DR,
                            )
                            nc.tensor.matmul(
                                px[:, j, 0:P], h1[:, 2, ms], w28[:, 2, :],
                                start=False, stop=False,
                            )
                            for kc, kn in ((0, 128), (1, 68)):
                                nc.tensor.matmul(
                                    px[:, j, 0:P], xb16[0:kn, kc, ms],
                                    i196[0:kn, kc, :],
                                    start=False, stop=False,
                                )
                            nc.tensor.matmul(
                                px[:, j, 0:P], ones[0:1, :], b2r[0:1, :],
                                start=False, stop=True,
                            )
                        for j in range(2):
                            mc = g * 2 + j
                            nc.scalar.activation(
                                x2T[:, mc, r0 : r0 + P], px[:, j, 0:P], AF.Copy,
                                scale=1.0 / 64.0,
                            )
                    nc.vector.tensor_mul(
                        x2sq[:, :, r0 : r0 + P],
                        x2T[:, :, r0 : r0 + P], x2T[:, :, r0 : r0 + P],
                    )
                    nc.vector.tensor_copy(
                        x2T8[:, :, r0 : r0 + P], x2T[:, :, r0 : r0 + P]
                    )

                x2rows_blk = {}

                def chan_norm(bi):
                    """row norms for block bi -> augmc row 1, plus the
                    shortcut transposes, issued a block early so the DMA
                    queues drain them in the shadow of compute."""
                    r0, rn, _ = blocks[bi]
                    nsub = (rn + 127) // 128
                    psx = ps_tile().rearrange("p a b -> p (a b)")
                    for kc in range(6):
                        nc.tensor.matmul(
                            psx[0:1, 0:rn], ones[:, 0:1],
                            x2sq[:, kc, r0 : r0 + rn],
                            start=(kc == 0), stop=(kc == 5),
                        )
                    xn16c = wp.tile([1, RB], F16, tag="xn16c")
                    nc.scalar.copy(xn16c[0:1, 0:rn], psx[0:1, 0:rn])
                    nc.gpsimd.dma_start(
                        augmc[1:2, r0 : r0 + rn], xn16c[0:1, 0:rn]
                    )
                    qeng = [nc.sync, nc.scalar]
                    xra = wp.tile([128, 6, RB], F16, tag="xra", bufs=2)
                    for s in range(nsub):
                        for kc in range(6):
                            qeng[(s * 6 + kc) % 2].dma_start_transpose(
                                xra[:, kc, s * 128 : s * 128 + 128],
                                x2T[:, kc, r0 + s * 128 : r0 + s * 128 + 128],
                            )
                    x2rows_blk[bi] = xra

                def chan_block(bi):
                    r0, rn, _ = blocks[bi]
                    nsub = (rn + 127) // 128
                    po = pp.tile([128, 4, 3, CH // 3], F32, tag="po", bufs=1)
                    h2all = wp.tile([128, 24, RB], F8, tag="h2all", bufs=1)

                    xra = x2rows_blk.pop(bi)

                    def ch_tail(g, pd, pdf, sq2):
                        for j in range(2):
                            mc = g * 2 + j
                            nc.tensor.matmul(
                                pdf[:, j * 512 : j * 512 + rn],
                                augwc[0:2, mc, :],
                                augmc[0:2, r0 : r0 + rn],
                                start=False, stop=True, skip_group_check=True,
                            )
                        rec2 = wp.tile([128, 2, RB], F32, tag="rec2", bufs=2)
                        if rn == RB:
                            nc.vector.reciprocal_approx_fast(
                                rec2.rearrange("p a b -> p (a b)"),
                                pdf[:, 0:1024],
                            )
                        else:
                            for j in range(2):
                                nc.vector.reciprocal_approx_fast(
                                    rec2[:, j, 0:rn],
                                    pdf[:, j * 512 : j * 512 + rn],
                                )
                        nc.gpsimd.tensor_mul(
                            h2all[:, 2 * g : 2 * g + 2, 0:rn],
                            sq2[:, 0:2, 0:rn], rec2[:, 0:2, 0:rn],
                        )

                    pend = None
                    for g in range(12):
                        pd = ps_tile()
                        pdf = pd.rearrange("p a b -> p (a b)")
                        for j in range(2):
                            mc = g * 2 + j
                            ms = slice(mc * 128, (mc + 1) * 128)
                            for kcp in range(3):
                                nc.tensor.matmul(
                                    pdf[:, j * 512 : j * 512 + rn],
                                    cwm2[:, 2 * kcp : 2 * kcp + 2, ms],
                                    x2T8[:, 2 * kcp : 2 * kcp + 2, r0 : r0 + rn],
                                    start=(kcp == 0), stop=(kcp == 2),
                                    perf_mode=DR,
                                )
                        sq2 = wp.tile([128, 2, RB], BF16, tag="sq2", bufs=3)
                        for j in range(2):
                            mc = g * 2 + j
                            nc.scalar.activation(
                                sq2[:, j, 0:rn],
                                pdf[:, j * 512 : j * 512 + rn], AF.Square,
                                bias=cbsc[:, mc : mc + 1], scale=sqsc[:, 1:2],
                            )
                        if pend is not None:
                            ch_tail(*pend)
                        pend = (g, pd, pdf, sq2)
                    ch_tail(*pend)

                    # two half-C h2w4 passes over the persistent h2
                    osbs = [wp.tile([128, 6, 128], F16, tag="osb", bufs=4,
                                    name="osb") for _ in range(nsub)]
                    for half in range(2):
                        c0 = half * CH
                        for g in range(12):
                            for s in range(nsub):
                                sn = min(128, rn - s * 128)
                                nc.tensor.matmul(
                                    po[0:sn, s, :, :],
                                    h2all[:, 2 * g : 2 * g + 2,
                                          s * 128 : s * 128 + sn],
                                    w48[:, 2 * g : 2 * g + 2, c0 : c0 + CH],
                                    start=(g == 0), stop=False, perf_mode=DR,
                                )
                        for s in range(nsub):
                            sn = min(128, rn - s * 128)
                            rs = r0 + s * 128
                            nc.tensor.matmul(
                                po[0:sn, s, :, :], ones[0:1, 0:sn],
                                b4r[0:1, c0 : c0 + CH],
                                start=False, stop=True,
                            )
                            nc.vector.scalar_tensor_tensor(
                                osbs[s][0:sn, 3 * half : 3 * half + 3, :],
                                po[0:sn, s, :, :], 2.0 ** -11,
                                xra[0:sn, 3 * half : 3 * half + 3,
                                    s * 128 : s * 128 + 128],
                                ALU.mult, ALU.add,
                            )
                            if half == 1:
                                nc.gpsimd.dma_start(
                                    out_dram[rs : rs + sn, :],
                                    osbs[s][0:sn, :, :]
                                    .rearrange("p a b -> p (a b)"),
                                )

                # emission order: stagger token A/B, channel norms early,
                # channel blocks as soon as their batches are done.
                next_norm = 0
                next_blk = 0

                def after_b(b):
                    nonlocal next_norm
                    while next_norm < len(blocks) and blocks[next_norm][2] <= b:
                        chan_norm(next_norm)
                        next_norm += 1

                def blocks_ready(b):
                    nonlocal next_blk
                    while next_blk < len(blocks) and blocks[next_blk][2] <= b:
                        chan_block(next_blk)
                        next_blk += 1

                token_a(0)
                for b in range(BL):
                    if b + 1 < BL:
                        token_a(b + 1)
                    token_b(b)
                    after_b(b)
                    if b >= 1:
                        blocks_ready(b - 1)
                blocks_ready(BL - 1)

    nc.compile()
    return nc


def _pack_kpn8(w, n_chunks, scale):
    """(K, N) fp32 -> (128, n_chunks, N) fp8 with zero K-padding."""
    k, n = w.shape
    out = np.zeros((n_chunks * 128, n), np.float32)
    out[:k] = w * scale
    return np.ascontiguousarray(
        out.reshape(n_chunks, 128, n).transpose(1, 0, 2)
    ).astype(NP8)


def _pack_col(v, n_chunks):
    out = np.zeros((n_chunks * 128,), np.float32)
    out[: v.shape[0]] = v
    return np.ascontiguousarray(out.reshape(n_chunks, 128).T)


_PROGRAM = None


def _get_program():
    global _PROGRAM
    if _PROGRAM is None:
        _PROGRAM = build_program()
    return _PROGRAM


def kernel(x, tw, tb, t_alpha, w2, b2, cw, cb, c_alpha, w4, b4, _trace=False):
    x = np.asarray(x, np.float32)
    tw = np.asarray(tw, np.float32)
    tb = np.asarray(tb, np.float32)
    w2 = np.asarray(w2, np.float32)
    b2 = np.asarray(b2, np.float32)
    cw = np.asarray(cw, np.float32)
    cb = np.asarray(cb, np.float32)
    w4 = np.asarray(w4, np.float32)
    b4 = np.asarray(b4, np.float32)

    s_t = np.float32(np.sqrt(np.float32(T / np.log(T + 1.0)))) ** np.asarray(
        t_alpha, np.float32
    )[0]
    s_c = np.float32(np.sqrt(np.float32(M3 / np.log(M3 + 1.0)))) ** np.asarray(
        c_alpha, np.float32
    )[0]
    g_t = np.float32(np.sqrt(64.0 * s_t))
    g_c = np.float32(np.sqrt(128.0 * s_c))

    wn_t = (tw ** 2).sum(1) + EPS
    wn_c = (cw ** 2).sum(1) + EPS
    augwt = np.zeros((2, 3, 128), np.float16)
    augwt[0].flat[:T] = wn_t.astype(np.float16)
    augwt[1] = 1.0
    augwc = np.zeros((2, 24, 128), np.float16)
    augwc[0].flat[:M3] = wn_c.astype(np.float16)
    augwc[1] = 1.0
    sqsc = np.zeros((128, 2), np.float32)
    sqsc[:, 0] = -0.5 * g_t
    sqsc[:, 1] = -0.5 * g_c

    shared = {
        "twm2": _pack_kpn8(tw.T, 2, -2.0),
        "w28": _pack_kpn8(w2.T, 3, 1.0),
        "i196": np.ascontiguousarray(
            np.pad(64.0 * np.eye(P, dtype=np.float32), ((0, 60), (0, 0)))
            .reshape(2, 128, P).transpose(1, 0, 2)).astype(np.float16),
        "b2r": (64.0 * b2).astype(np.float16).reshape(1, P),
        "augwt": augwt,
        "cwm2": _pack_kpn8(cw.T, 6, -2.0),
        "w48": _pack_kpn8(w4.T, 24, 16.0),
        "b4r": (2048.0 * b4).astype(np.float16).reshape(1, C),
        "augwc": augwc,
        "tbsc": _pack_col(g_t * tb, 3),
        "cbsc": _pack_col(g_c * cb, 24),
        "sqsc": sqsc,
    }
    xr = x.reshape(NCORES, BL, P, C)
    x16 = np.zeros((NCORES, BL, 128, 2, C), np.float16)
    x16[:, :, :, 0] = xr[:, :, 0:128]
    x16[:, :, 0:68, 1] = xr[:, :, 128:P]
    x8 = np.zeros((NCORES, BL, 128, 2, C), NP8)
    x8[:, :, :, 0] = xr[:, :, 0:128].astype(NP8)
    x8[:, :, 0:68, 1] = xr[:, :, 128:P].astype(NP8)
    in_maps = [dict(shared, x16=x16[c], x8=x8[c]) for c in range(NCORES)]

    nc = _get_program()
    kwargs = {}
    if _trace:
        import os
        import shutil

        shutil.rmtree("/tmp/bass_ntff", ignore_errors=True)
        os.makedirs("/tmp/bass_ntff", exist_ok=True)
        kwargs["tmpdir"] = "/tmp/bass_ntff"
    res = bass_utils.run_bass_kernel_spmd(
        nc, in_maps, core_ids=list(range(NCORES)), trace=_trace, **kwargs
    )
    out = np.concatenate(
        [np.asarray(res.results[c]["out"]) for c in range(NCORES)], axis=0
    )
    out = out.reshape(B, P, C).astype(np.float32)
    if _trace:
        kernel.last_results = res
    return out


# revision 24
# speedup vs baseline: 1.0516x; 1.0110x over previous
"""Trainium2 Bass kernel for the YAT MixerBlock (nn_MixerBlock_12524124635797).

Strategy: pure data-parallel over batch (64 -> 8 per core). Each core runs
the full mixer block for its 8 batch elements.

Per-core dataflow (all GEMMs fp16 inputs, fp32 PSUM accumulation):
  Token stage (per batch b, x_b is (196p, 768c)):
    dot1 (384t-part, 768c-free) = twT.T @ x_b            [PE]
    den  = wn_t[t] + xn[c] - 2*dot1 + eps                [DVE affine_then_add]
    rec  = 1/den                                         [DVE reciprocal_approx_fast]
    sq   = (dot1 + tb[t])^2                              [ACT Square, bias slot]
    h1   = sq * rec  (fp16)                              [GPSIMD mult; scale_t folded into w2]
    x2T (768c-part, 196p-free) = h1.T@w2sT + x_b.T@I196 + ones.T@b2row   [PE, shortcut+bias
                                                          folded in as extra K rows]
  Channel stage (rows = (b,p) flattened, 1568 per core):
    xn2b (128, rows) = ones.T @ (x2T*x2T)                [PE broadcast of row norms]
    for row-block rb, for m-chunk mc (24 chunks of 3072):
      dot2 (128m-part, rows-free) = cwT.T @ x2T          [PE]
      den2/rec2/sq2/h2 as above (wn_c, cb per-partition) [DVE/ACT/GPSIMD]
      out_psum(rows-part, 768c) += h2.T @ w4sT[mc]       [PE]
    out_psum += x2T.T @ I768 + ones.T @ b4row            [PE, shortcut+bias]
    out (rows, 768) fp32 -> DRAM                         [ACT copy + DMA]
"""

import numpy as np

import concourse.bass as bass
import concourse.bacc as bacc
import concourse.mybir as mybir
from concourse import bass_utils
from concourse import tile

F16 = mybir.dt.float16
F32 = mybir.dt.float32
AF = mybir.ActivationFunctionType

EPS = 0.1
B, P, C, T, M3 = 64, 196, 768, 384, 3072
NCORES = 8
BL = B // NCORES          # 8 batches per core
ROWS = BL * P             # 1568 rows per core
ROWSP = 1664              # ROWS padded to a multiple of 128
RB = 256                  # row-block size for the channel stage (2 psum chunks)


def _ceil_div(a, b):
    return (a + b - 1) // b


def _n_slices(n, step=512):
    """Split [0, n) into matmul-legal free-dim slices (<=512, bank-aligned)."""
    out = []
    o = 0
    while o < n:
        out.append((o, min(step, n - o)))
        o += step
    return out


def build_program():
    nc = bacc.Bacc(
        "TRN2",
        target_bir_lowering=False,
        debug=False,
        enable_asserts=False,
        num_devices=NCORES,
    )

    # ---- DRAM I/O ----
    d = {}
    d["xa"] = nc.dram_tensor("xa", [BL, 128, C], F16, kind="ExternalInput").ap()
    d["xb"] = nc.dram_tensor("xb", [BL, 128, C], F16, kind="ExternalInput").ap()
    d["twT"] = nc.dram_tensor("twT", [128, 2, T], F16, kind="ExternalInput").ap()
    d["w2sT"] = nc.dram_tensor("w2sT", [128, 3, P], F16, kind="ExternalInput").ap()
    d["i196"] = nc.dram_tensor("i196", [128, 2, P], F16, kind="ExternalInput").ap()
    d["b2r"] = nc.dram_tensor("b2r", [1, P], F16, kind="ExternalInput").ap()
    d["cwT"] = nc.dram_tensor("cwT", [128, 6, M3], F16, kind="ExternalInput").ap()
    d["w4sT"] = nc.dram_tensor("w4sT", [128, 24, C], F16, kind="ExternalInput").ap()
    d["b4r"] = nc.dram_tensor("b4r", [1, C], F16, kind="ExternalInput").ap()
    d["wnt"] = nc.dram_tensor("wnt", [128, 3], F32, kind="ExternalInput").ap()
    d["tbc"] = nc.dram_tensor("tbc", [128, 3], F32, kind="ExternalInput").ap()
    d["wnc"] = nc.dram_tensor("wnc", [128, 24], F32, kind="ExternalInput").ap()
    d["cbc"] = nc.dram_tensor("cbc", [128, 24], F32, kind="ExternalInput").ap()
    out_dram = nc.dram_tensor("out", [ROWS, C], F32, kind="ExternalOutput").ap()

    with tile.TileContext(nc) as tc:
        with tc.tile_pool(name="consts", bufs=1) as cp:
            # Resident constants / persistent activations.
            twT = cp.tile([128, 2, T], F16)
            w2sT = cp.tile([128, 3, P], F16)
            i196 = cp.tile([128, 2, P], F16)
            b2r = cp.tile([128, P], F16)
            cwT = cp.tile([128, 6, M3], F16)
            w4sT = cp.tile([128, 24, C], F16)
            b4r = cp.tile([128, C], F16)
            wnt = cp.tile([128, 3], F32)
            tbc = cp.tile([128, 3], F32)
            wnc = cp.tile([128, 24], F32)
            cbc = cp.tile([128, 24], F32)
            ones = cp.tile([128, 128], F16)
            # Free dim padded to a multiple of 128 so the tail row-block's
            # 128-col DMA transpose reads stay in bounds (garbage cols unused).
            x2T = cp.tile([128, 6, ROWSP], F16)
            xn2b = cp.tile([128, ROWS], F32)

            # x input first (token stage's critical path) as two big strided
            # DMAs, then small token constants, all on the sync queue; the big
            # channel weights go on the scalar-engine HWDGE queue so they
            # don't block the token stage.
            # Per-batch x tiles: separate tiles so batch 0's consumers only
            # wait on batch 0's DMA. Startup-critical loads go first on sync;
            # big channel weights on the scalar queue.
            xbs = []
            nc.sync.dma_start(twT[:], d["twT"])
            for b in range(BL):
                xb = cp.tile([128, 2, C], F16, name=f"xb{b}")
                nc.sync.dma_start(xb[:, 0, :], d["xa"][b])
                nc.sync.dma_start(xb[0:68, 1, :], d["xb"][b, 0:68, :])
                xbs.append(xb)
                if b == 0:
                    nc.sync.dma_start(w2sT[:], d["w2sT"])
                    nc.sync.dma_start(i196[:], d["i196"])
                    nc.sync.dma_start(b2r[0:1, :], d["b2r"])
                    nc.sync.dma_start(wnt[:], d["wnt"])
                    nc.sync.dma_start(tbc[:], d["tbc"])
            nc.sync.dma_start(wnc[:], d["wnc"])
            nc.sync.dma_start(cbc[:], d["cbc"])
            nc.scalar.dma_start(cwT[:], d["cwT"])
            nc.scalar.dma_start(w4sT[:], d["w4sT"])
            nc.scalar.dma_start(b4r[0:1, :], d["b4r"])
            nc.vector.memset(ones[:], 1.0)
            nc.vector.memset(x2T[:, :, ROWS:ROWSP], 0.0)

            # ================= Token stage =================
            with (
                tc.tile_pool(name="tok_sbuf", bufs=2) as tp,
                tc.tile_pool(name="tok_psum", bufs=1, space="PSUM") as pp,
            ):
                for b in range(BL):
                    r0 = b * P
                    xb = xbs[b]

                    # dot1 first: it only needs twT + x, so the PE can start
                    # before the norm chain is ready.
                    dot1s = []
                    for tcn in range(3):
                        ps_dot1 = pp.tile(
                            [128, C], F32, tag="ps_dot1", bufs=2, name="ps_dot1"
                        )
                        for kc, kn in ((0, 128), (1, 68)):
                            for no, nn_ in _n_slices(C):
                                nc.tensor.matmul(
                                    ps_dot1[:, no : no + nn_],
                                    twT[0:kn, kc, tcn * 128 : (tcn + 1) * 128],
                                    xb[0:kn, kc, no : no + nn_],
                                    start=(kc == 0),
                                    stop=(kc == 1),
                                )
                        dot1s.append(ps_dot1)

                    # x-norm broadcast tile: xnb[q, c] = sum_p x[p, c]^2
                    xsq = tp.tile([128, 2, C], F16, tag="xsq")
                    nc.vector.tensor_mul(xsq[:, 0, :], xb[:, 0, :], xb[:, 0, :])
                    nc.vector.tensor_mul(
                        xsq[0:68, 1, :], xb[0:68, 1, :], xb[0:68, 1, :]
                    )
                    ps_xnb = pp.tile([128, C], F32, tag="ps_xnb", bufs=1)
                    for no, nn_ in _n_slices(C):
                        nc.tensor.matmul(
                            ps_xnb[:, no : no + nn_],
                            ones[:, :],
                            xsq[:, 0, no : no + nn_],
                            start=True,
                            stop=False,
                        )
                        nc.tensor.matmul(
                            ps_xnb[:, no : no + nn_],
                            ones[0:68, :],
                            xsq[0:68, 1, no : no + nn_],
                            start=False,
                            stop=True,
                        )
                    xnb = tp.tile([128, C], F32, tag="xnb")
                    nc.scalar.copy(xnb[:], ps_xnb[:])

                    h1 = tp.tile([128, 3, C], F16, tag="h1")
                    for tcn in range(3):
                        ps_dot1 = dot1s[tcn]
                        den = tp.tile([128, C], F32, tag="den")
                        nc.vector.affine_then_add(
                            den[:], ps_dot1[:], xnb[:],
                            scale=-2.0, bias=wnt[:, tcn : tcn + 1],
                        )
                        rec = tp.tile([128, C], F32, tag="rec")
                        nc.vector.reciprocal_approx_fast(rec[:], den[:])
                        sq = tp.tile([128, C], F32, tag="sq")
                        nc.scalar.activation(
                            sq[:], ps_dot1[:], AF.Square, bias=tbc[:, tcn : tcn + 1]
                        )
                        nc.gpsimd.tensor_mul(h1[:, tcn, :], sq[:], rec[:])

                    # token linear + shortcut + bias -> x2T columns for batch b
                    for mc in range(6):
                        ps_x2 = pp.tile([128, P], F32, tag="ps_x2", bufs=2)
                        for kc in range(3):
                            nc.tensor.matmul(
                                ps_x2[:],
                                h1[:, kc, mc * 128 : (mc + 1) * 128],
                                w2sT[:, kc, :],
                                start=(kc == 0),
                                stop=False,
                            )
                        for kc, kn in ((0, 128), (1, 68)):
                            nc.tensor.matmul(
                                ps_x2[:],
                                xb[0:kn, kc, mc * 128 : (mc + 1) * 128],
                                i196[0:kn, kc, :],
                                start=False,
                                stop=False,
                            )
                        nc.tensor.matmul(
                            ps_x2[:],
                            ones[0:1, :],
                            b2r[0:1, :],
                            start=False,
                            stop=True,
                        )
                        nc.scalar.copy(x2T[:, mc, r0 : r0 + P], ps_x2[:])

            # ================= Channel-stage row norms =================
            with (
                tc.tile_pool(name="xn_sbuf", bufs=1) as xp,
                tc.tile_pool(name="xn_psum", bufs=1, space="PSUM") as xpp,
            ):
                ps_xn2 = xpp.tile([128, ROWS], F32)
                for kc in range(6):
                    x2sq = xp.tile([128, ROWS], F16, tag="x2sq", bufs=2)
                    nc.vector.tensor_mul(x2sq[:], x2T[:, kc, 0:ROWS], x2T[:, kc, 0:ROWS])
                    for no, nn_ in _n_slices(ROWS):
                        nc.tensor.matmul(
                            ps_xn2[:, no : no + nn_],
                            ones[:, :],
                            x2sq[:, no : no + nn_],
                            start=(kc == 0),
                            stop=(kc == 5),
                        )
                nc.scalar.copy(xn2b[:], ps_xn2[:])

            # ================= Channel stage =================
            with (
                tc.tile_pool(name="ch_sbuf", bufs=2) as chp,
                tc.tile_pool(name="ch_psum", bufs=1, space="PSUM") as cpp,
            ):
                for r0 in range(0, ROWS, RB):
                    rn = min(RB, ROWS - r0)
                    nsub = _ceil_div(rn, 128)
                    po = [
                        cpp.tile([128, C], F32, tag=f"po{s}", bufs=1, name=f"po{s}")
                        for s in range(nsub)
                    ]
                    for mc in range(24):
                        ps_d2 = cpp.tile([128, RB], F32, tag="ps_d2", bufs=4)
                        for kc in range(6):
                            nc.tensor.matmul(
                                ps_d2[:, 0:rn],
                                cwT[:, kc, mc * 128 : (mc + 1) * 128],
                                x2T[:, kc, r0 : r0 + rn],
                                start=(kc == 0),
                                stop=(kc == 5),
                            )
                        den2 = chp.tile([128, RB], F32, tag="den2", bufs=4)
                        nc.vector.affine_then_add(
                            den2[:, 0:rn], ps_d2[:, 0:rn], xn2b[:, r0 : r0 + rn],
                            scale=-2.0, bias=wnc[:, mc : mc + 1],
                        )
                        rec2 = chp.tile([128, RB], F32, tag="rec2", bufs=4)
                        nc.vector.reciprocal_approx_fast(rec2[:, 0:rn], den2[:, 0:rn])
                        sq2 = chp.tile([128, RB], F32, tag="sq2", bufs=4)
                        nc.scalar.activation(
                            sq2[:, 0:rn], ps_d2[:, 0:rn], AF.Square,
                            bias=cbc[:, mc : mc + 1],
                        )
                        h2 = chp.tile([128, RB], F16, tag="h2", bufs=4)
                        # Alternate the multiply between GPSIMD and DVE so the
                        # last link of the yat chain isn't serialized on one
                        # engine's FIFO.
                        mul_eng = nc.gpsimd if mc % 3 else nc.vector
                        mul_eng.tensor_mul(h2[:, 0:rn], sq2[:, 0:rn], rec2[:, 0:rn])

                        for s in range(nsub):
                            sn = min(128, rn - s * 128)
                            for no, nn_ in _n_slices(C):
                                nc.tensor.matmul(
                                    po[s][0:sn, no : no + nn_],
                                    h2[:, s * 128 : s * 128 + sn],
                                    w4sT[:, mc, no : no + nn_],
                                    start=(mc == 0),
                                    stop=False,
                                )
                    # bias b4 row, then shortcut x2 added via DVE from a
                    # DMA-transposed copy of x2T (cheaper than routing the
                    # identity through the PE).
                    for s in range(nsub):
                        sn = min(128, rn - s * 128)
                        rs = r0 + s * 128
                        for no, nn_ in _n_slices(C):
                            nc.tensor.matmul(
                                po[s][0:sn, no : no + nn_],
                                ones[0:1, 0:sn],
                                b4r[0:1, no : no + nn_],
                                start=False,
                                stop=True,
                            )
                        x2row = chp.tile([128, 6, 128], F16, tag="x2row", bufs=3)
                        for kc in range(6):
                            # Always a full 128-col source block (x2T free dim
                            # is padded); extra rows of x2row are unused.
                            nc.sync.dma_start_transpose(
                                x2row[:, kc, :], x2T[:, kc, rs : rs + 128]
                            )
                        osb = chp.tile([128, C], F32, tag="osb", bufs=3)
                        nc.vector.tensor_add(
                            osb[0:sn, :],
                            po[s][0:sn, :],
                            x2row[0:sn, :, :].rearrange("p a b -> p (a b)"),
                        )
                        nc.sync.dma_start(out_dram[rs : rs + sn, :], osb[0:sn, :])

    nc.compile()
    return nc


def _pack_kpn(w, n_chunks):
    """(K, N) fp32 -> (128, n_chunks, N) fp16 with zero padding of K."""
    k, n = w.shape
    out = np.zeros((n_chunks * 128, n), np.float16)
    out[:k] = w.astype(np.float16)
    return np.ascontiguousarray(
        out.reshape(n_chunks, 128, n).transpose(1, 0, 2)
    )


def _pack_col(v, n_chunks):
    """(K,) fp32 -> (128, n_chunks) fp32 column chunks."""
    out = np.zeros((n_chunks * 128,), np.float32)
    out[: v.shape[0]] = v.astype(np.float32)
    return np.ascontiguousarray(out.reshape(n_chunks, 128).T)


_PROGRAM = None


def _get_program():
    global _PROGRAM
    if _PROGRAM is None:
        _PROGRAM = build_program()
    return _PROGRAM


def kernel(x, tw, tb, t_alpha, w2, b2, cw, cb, c_alpha, w4, b4, _trace=False):
    x = np.asarray(x, np.float32)
    tw = np.asarray(tw, np.float32)
    tb = np.asarray(tb, np.float32)
    w2 = np.asarray(w2, np.float32)
    b2 = np.asarray(b2, np.float32)
    cw = np.asarray(cw, np.float32)
    cb = np.asarray(cb, np.float32)
    w4 = np.asarray(w4, np.float32)
    b4 = np.asarray(b4, np.float32)

    # YAT output scales (exactly as the reference computes them), folded into
    # the following linear layers' weights and biases' stays separate.
    scale_t = np.float32(np.sqrt(np.float32(T / np.log(T + 1.0)))) ** np.asarray(
        t_alpha, np.float32
    )[0]
    scale_c = np.float32(np.sqrt(np.float32(M3 / np.log(M3 + 1.0)))) ** np.asarray(
        c_alpha, np.float32
    )[0]
    w2s = (w2 * scale_t).astype(np.float32)   # (P, T)
    w4s = (w4 * scale_c).astype(np.float32)   # (C, M3)

    shared = {
        "twT": _pack_kpn(tw.T, 2),                       # (196,384) -> (128,2,384)
        "w2sT": _pack_kpn(w2s.T, 3),                     # (384,196) -> (128,3,196)
        "i196": _pack_kpn(np.eye(P, dtype=np.float32), 2),
        "b2r": b2.astype(np.float16).reshape(1, P),
        "cwT": _pack_kpn(cw.T, 6),                       # (768,3072)
        "w4sT": _pack_kpn(w4s.T, 24),                    # (3072,768)
        "b4r": b4.astype(np.float16).reshape(1, C),
        "wnt": _pack_col((tw.astype(np.float32) ** 2).sum(1) + EPS, 3),
        "tbc": _pack_col(tb, 3),
        "wnc": _pack_col((cw.astype(np.float32) ** 2).sum(1) + EPS, 24),
        "cbc": _pack_col(cb, 24),
    }
    x16 = x.astype(np.float16).reshape(NCORES, BL, P, C)
    xa = np.ascontiguousarray(x16[:, :, 0:128, :])
    xbp = np.zeros((NCORES, BL, 128, C), np.float16)
    xbp[:, :, 0:68] = x16[:, :, 128:P, :]
    in_maps = [dict(shared, xa=xa[c], xb=xbp[c]) for c in range(NCORES)]

    nc = _get_program()
    kwargs = {}
    if _trace:
        import shutil

        shutil.rmtree("/tmp/bass_ntff", ignore_errors=True)
        import os

        os.makedirs("/tmp/bass_ntff", exist_ok=True)
        kwargs["tmpdir"] = "/tmp/bass_ntff"
    res = bass_utils.run_bass_kernel_spmd(
        nc, in_maps, core_ids=list(range(NCORES)), trace=_trace, **kwargs
    )
    out = np.concatenate([res.results[c]["out"] for c in range(NCORES)], axis=0)
    out = out.reshape(B, P, C).astype(np.float32)
    if _trace:
        kernel.last_results = res
    return out
